# revision 6
# baseline (speedup 1.0000x reference)
"""Row-wise cosine-similarity loss (1 - mean(cos)) for N=16384, D=2048.

Levers vs the f32 DVE/ACT baseline (93 us, at the f32 DMA roofline):

1. fp8-e4m3 inputs.  The loss tolerance (rel 2e-2 on a value ~1.0 with
   mean(cos) ~ 2e-4) leaves orders of magnitude of headroom; e4m3
   quantization measures rel-err ~3e-6.  HBM traffic drops 4x: 8.4 MB
   per core, ~25 us at the ~330 GB/s per-core DMA roofline.

2. Tensor-engine reductions.  At fp8 the DVE/ACT elementwise engines
   run at 1 elem/cycle/partition (no 2x mode below 2-byte dtypes), so
   the three per-row reductions (a.b, a.a, b.b) would cost ~49 us on
   DVE+ACT — twice the DMA floor.  Instead the host pre-transposes each
   core's rows into D-major blocks and the PE contracts D in fp8
   DoubleRow mode (256 contraction elements per pass): for each block
   of 128 rows, matmuls accumulate Gram tiles in PSUM whose diagonals
   are the per-row terms.  DVE extracts diagonals with an identity-mask
   multiply-accumulate, then the usual rsqrt/mult gives the cosines.

3. Interleaved a/b layout.  The host interleaves both tensors into one
   dram tensor ab[rb, p, k, t, r] (t = which tensor), so a single
   stationary load of a's block serves a 512-row moving stream
   ([a | b] -> aT.a and aT.b Gram tiles side by side in one PSUM bank),
   and b's block serves bT.b.  That is 2 stationary loads + 3 Gram
   products per (row-block, k-pair): PE weight-load traffic (which does
   not overlap the matmul stream on this hardware) drops by a third vs
   the naive 3-group form, and one DMA per row-block (4 KiB/partition
   contiguous) halves descriptor-generation load.

Data-parallel across 8 NeuronCores (2048 rows each); host averages the
8x[128,16] cosine tiles into the scalar loss.

The walrus build in this container accepts at most ONE semaphore wait
per instruction; Tile emits several.  _split_multi_waits() post-passes
the BIR and hoists extra waits onto NOPs inserted just before the
offending instruction on the same engine.
"""

import numpy as np
import ml_dtypes

N, D = 16384, 2048
NCORES = 8
NS = N // NCORES  # rows per core
P = 128  # SBUF partitions / PE contraction width
T = NS // P  # row-blocks per core (16)
K = D // P  # contraction slots (16); processed as 8 DoubleRow pairs
KP = K // 2
BUFS = 3  # row-block chunk buffering

_cached_nc = None


def _split_multi_waits(nc):
    """Walrus here supports one sem-wait per instruction; split extras
    onto NOPs inserted immediately before, on the same engine."""
    import concourse.mybir as mybir

    n = 0
    for f in nc.m.functions:
        for bb in f.blocks:
            insts = bb.instructions
            out = []
            changed = False
            for ins in insts:
                si = getattr(ins, "sync_info", None)
                ow = list(si.on_wait) if si is not None and si.on_wait else []
                if len(ow) > 1:
                    changed = True
                    for w in ow[:-1]:
                        n += 1
                        out.append(
                            mybir.InstNoOp(
                                name=f"{ins.name}-wsplit{n}",
                                engine=ins.engine,
                                bass_nofuse=True,
                                sync_info=mybir.SyncInfo(
                                    on_wait=[w], on_update=[]
                                ),
                            )
                        )
                    si.on_wait = [ow[-1]]
                out.append(ins)
            if changed:
                bb.instructions = out
    return n


def _build(reps=1, hw_loop=False):
    """hw_loop=True wraps the reps in a tc.For_i hardware loop (compact
    NEFF for timing); reps are python-unrolled otherwise."""
    import contextlib

    import concourse.bass as bass
    import concourse.mybir as mybir
    import concourse.tile as tile

    f32 = mybir.dt.float32
    f8 = mybir.dt.float8e4
    Alu = mybir.AluOpType
    DR = mybir.MatmulPerfMode.DoubleRow

    nc = bass.Bass("TRN2", target_bir_lowering=False)
    ab = nc.dram_tensor("ab", [NS, 2 * D], f8, kind="ExternalInput")
    eye_d = nc.dram_tensor("eye", [P, P], f32, kind="ExternalInput")
    out = nc.dram_tensor("cos", [P, T], f32, kind="ExternalOutput")

    # dram row = rb*128 + p; col = ((k*2) + t)*128 + r  (host pre-blocked;
    # t selects tensor a/b)
    abv = ab.rearrange("(rb p) (k t r) -> rb p k t r", p=P, k=K, t=2)

    with tile.TileContext(nc) as tc:
        with (
            tc.tile_pool(name="abpool", bufs=BUFS) as abpool,
            tc.tile_pool(name="psum_ad", bufs=2, space="PSUM") as pad,
            tc.tile_pool(name="psum_nb", bufs=2, space="PSUM") as pnb,
            tc.tile_pool(name="singles", bufs=1) as singles,
            tc.tile_pool(name="small", bufs=2) as small,
        ):
            eye = singles.tile([P, P], f32, tag="eye")
            nc.sync.dma_start(out=eye, in_=eye_d[:])
            dot_buf = singles.tile([P, T], f32, tag="dot")
            na_buf = singles.tile([P, T], f32, tag="na")
            nb_buf = singles.tile([P, T], f32, tag="nb")
            cos_buf = singles.tile([P, T], f32, tag="cos")
            scr = singles.tile([P, P], f32, tag="scr")

            def diag(dst, psum):
                # dst[p] = sum_f psum[p, f] * eye[p, f] = psum[p, p]
                nc.vector.scalar_tensor_tensor(
                    out=scr,
                    in0=psum,
                    scalar=1.0,
                    in1=eye,
                    op0=Alu.mult,
                    op1=Alu.mult,
                    accum_out=dst,
                )

            if hw_loop and reps > 1:
                rep_ctx = tc.For_i(0, reps)
                rep_range = [0]
            else:
                rep_ctx = contextlib.nullcontext()
                rep_range = range(reps)

            with rep_ctx:
              for _rep in rep_range:
                for rb in range(T):
                    abt = abpool.tile([P, K, 2, P], f8, tag="ab")
                    nc.sync.dma_start(out=abt, in_=abv[rb])
                    # Full-bank psum tiles so each accumulation group owns
                    # its 2 KiB zero-region.
                    ps_ad = pad.tile([P, 512], f32, tag="ad")
                    ps_nb = pnb.tile([P, 512], f32, tag="nb")
                    for kp in range(KP):
                        sa = abt[:, 2 * kp : 2 * kp + 2, 0, :]
                        sb = abt[:, 2 * kp : 2 * kp + 2, 1, :]
                        sab = abt[:, 2 * kp : 2 * kp + 2, :, :]
                        first, last = kp == 0, kp == KP - 1
                        # [aT.a | aT.b] in one pass off a single stationary
                        nc.tensor.matmul(
                            ps_ad[:, 0 : 2 * P],
                            sa,
                            sab,
                            start=first,
                            stop=last,
                            perf_mode=DR,
                        )
                        nc.tensor.matmul(
                            ps_nb[:, 0:P], sb, sb, start=first, stop=last, perf_mode=DR
                        )
                    diag(na_buf[:, rb : rb + 1], ps_ad[:, 0:P])
                    diag(dot_buf[:, rb : rb + 1], ps_ad[:, P : 2 * P])
                    diag(nb_buf[:, rb : rb + 1], ps_nb[:, 0:P])

            # cos = dot / sqrt(na*nb), batched over all T columns
            prod = small.tile([P, T], f32, tag="prod")
            nc.vector.tensor_mul(prod, na_buf, nb_buf)
            rs = small.tile([P, T], f32, tag="rs")
            nc.scalar.sqrt(rs, prod)
            rr = small.tile([P, T], f32, tag="rr")
            nc.vector.reciprocal(rr, rs)
            nc.vector.tensor_mul(cos_buf, dot_buf, rr)
            nc.sync.dma_start(out=out[:], in_=cos_buf)

    _split_multi_waits(nc)
    return nc


def _get_nc():
    global _cached_nc
    if _cached_nc is None:
        _cached_nc = _build()
    return _cached_nc


def _run(in_maps, **kwargs):
    from concourse.bass_utils import run_bass_kernel_spmd

    return run_bass_kernel_spmd(_get_nc(), in_maps, core_ids=list(range(NCORES)), **kwargs)


def _interleave(xa, xb):
    """Two [2048 rows, 2048 D] fp8 blocks -> [2048, 4096] with
    row' = rb*128 + p, col = (k*2 + t)*128 + r."""
    # rows = rb*128 + r, cols (D) = k*128 + p
    xa = xa.reshape(T, P, K, P)  # [rb, r, k, p]
    xb = xb.reshape(T, P, K, P)
    x = np.stack([xa, xb], axis=3)  # [rb, r, k, t, p]
    x = np.ascontiguousarray(x.transpose(0, 4, 2, 3, 1))  # [rb, p, k, t, r]
    return x.reshape(NS, 2 * D)


def _make_in_maps(cxr, ehr):
    cxr = np.asarray(cxr, dtype=np.float32).astype(ml_dtypes.float8_e4m3)
    ehr = np.asarray(ehr, dtype=np.float32).astype(ml_dtypes.float8_e4m3)
    eye = np.eye(P, dtype=np.float32)
    return [
        {
            "ab": _interleave(
                ehr[i * NS : (i + 1) * NS], cxr[i * NS : (i + 1) * NS]
            ),
            "eye": eye,
        }
        for i in range(NCORES)
    ]


def _combine(results):
    cos = np.stack([r["cos"] for r in results])  # [8, 128 p, 16 rb]
    return np.float32(1.0 - cos.astype(np.float64).mean())


def kernel(cxr, ehr):
    res = _run(_make_in_maps(cxr, ehr))
    return _combine(res.results)


# revision 7
# speedup vs baseline: 1.6655x; 1.6655x over previous
"""Row-wise cosine-similarity loss (1 - mean(cos)) for N=16384, D=2048 f32.

Levers vs the f32 DVE/ACT baseline (93 us, at the f32 DMA roofline):

1. fp8-e4m3 inputs.  The loss tolerance (rel 2e-2 on a value ~1.0 with
   mean(cos) ~ 2e-4) leaves orders of magnitude of precision headroom;
   e4m3 quantization of the inputs measures rel-err ~3e-6 on the loss.
   HBM traffic drops 4x: 8.4 MB per core, ~27 us at the measured
   ~310 GB/s per-core DMA rate.

2. Tensor-engine reductions.  At fp8 the DVE/ACT elementwise engines
   run at 1 elem/cycle/partition (2x modes need 2-byte dtypes), so the
   three per-row reductions (a.b, a.a, b.b) would cost ~49 us on
   DVE+ACT — twice the DMA floor.  Instead the host pre-transposes most
   row-blocks into D-major layout and the PE contracts D in fp8
   DoubleRow mode (256 contraction elements per pass): per 128-row
   block, one stationary load of a's k-pair slice serves a 512-row
   moving stream [a|b] producing [aT.a | aT.b] Gram tiles in one PSUM
   bank, and b's slice serves bT.b.  Diagonals of the accumulated Gram
   tiles are the per-row reduction values; DVE extracts them with an
   identity-mask multiply-accumulate.  Measured PE cost ~2.2-2.6 us per
   row-block (weight loads serialize with streams on this hardware).

3. Hybrid row-block split.  Pure-PE is PE-bound (~34-37 us), so RM=4 of
   the 16 row-blocks per core go down a row-major path instead: DVE
   computes the dot (fused multiply-reduce) and ACT the two squares.
   That shifts ~4 blocks of PE work onto otherwise-idle engines,
   balancing PE ~26 us / ACT ~17 us / DVE ~18 us against the ~27 us
   DMA stream (best measured: 29.5 us, 3.2x over the f32 baseline).

Data-parallel across 8 NeuronCores (2048 rows each); the host averages
the 8x[128,16] cosine tiles into the scalar loss.

The walrus build in this container accepts at most ONE semaphore wait
per instruction; Tile emits several.  _split_multi_waits() post-passes
the BIR and hoists extra waits onto NOPs inserted just before the
offending instruction on the same engine.
"""

import numpy as np
import ml_dtypes

N, D = 16384, 2048
NCORES = 8
NS = N // NCORES  # rows per core
P = 128  # SBUF partitions / PE contraction width
T = NS // P  # row-blocks per core (16)
K = D // P  # contraction slots (16); 8 DoubleRow pairs
KP = K // 2
RM = 4  # row-blocks on the DVE/ACT row-major path
T_PE = T - RM
BUFS = 4  # input chunk buffering
PSUM_BUFS = 3  # PSUM group double+ buffering (6 of 8 banks)

# Interleave rm blocks among pe blocks (every ~3rd) so DVE/ACT work
# overlaps the PE stream instead of clustering.
_ORDER = []
_pe, _rm = 0, 0
for _i in range(T):
    if _rm < RM and (_i % 3 == 2 or _pe >= T_PE):
        _ORDER.append(("rm", _rm))
        _rm += 1
    else:
        _ORDER.append(("pe", _pe))
        _pe += 1

_cached_nc = None


def _split_multi_waits(nc):
    """Walrus here supports one sem-wait per instruction; split extras
    onto NOPs inserted immediately before, on the same engine."""
    import concourse.mybir as mybir

    n = 0
    for f in nc.m.functions:
        for bb in f.blocks:
            insts = bb.instructions
            out = []
            changed = False
            for ins in insts:
                si = getattr(ins, "sync_info", None)
                ow = list(si.on_wait) if si is not None and si.on_wait else []
                if len(ow) > 1:
                    changed = True
                    for w in ow[:-1]:
                        n += 1
                        out.append(
                            mybir.InstNoOp(
                                name=f"{ins.name}-wsplit{n}",
                                engine=ins.engine,
                                bass_nofuse=True,
                                sync_info=mybir.SyncInfo(
                                    on_wait=[w], on_update=[]
                                ),
                            )
                        )
                    si.on_wait = [ow[-1]]
                out.append(ins)
            if changed:
                bb.instructions = out
    return n


def _build(reps=1, hw_loop=False):
    """hw_loop=True wraps the reps in a tc.For_i hardware loop (compact
    NEFF for timing); reps are python-unrolled otherwise."""
    import contextlib

    import concourse.bass as bass
    import concourse.mybir as mybir
    import concourse.tile as tile

    f32 = mybir.dt.float32
    f8 = mybir.dt.float8e4
    Alu = mybir.AluOpType
    Act = mybir.ActivationFunctionType
    DR = mybir.MatmulPerfMode.DoubleRow

    nc = bass.Bass("TRN2", target_bir_lowering=False)
    ab = nc.dram_tensor("ab", [T_PE * P, 2 * D], f8, kind="ExternalInput")
    rmd = nc.dram_tensor("rm", [RM * P, 2 * D], f8, kind="ExternalInput")
    eye_d = nc.dram_tensor("eye", [P, P], f32, kind="ExternalInput")
    out = nc.dram_tensor("cos", [P, T], f32, kind="ExternalOutput")

    # PE layout: dram row = rb*128 + p, col = (k*2 + t)*128 + r
    # (t selects tensor: 0 = ehr, 1 = cxr).
    abv = ab.rearrange("(rb p) (k t r) -> rb p k t r", p=P, k=K, t=2)
    # Row-major layout: dram row = rb*128 + p (natural rows), col = t*D + d.
    rmv = rmd.rearrange("(rb p) (t d) -> rb p t d", p=P, t=2)

    with tile.TileContext(nc) as tc:
        with (
            tc.tile_pool(name="abpool", bufs=BUFS) as abpool,
            tc.tile_pool(name="rmpool", bufs=BUFS) as rmpool,
            tc.tile_pool(name="psum_ad", bufs=PSUM_BUFS, space="PSUM") as pad,
            tc.tile_pool(name="psum_nb", bufs=PSUM_BUFS, space="PSUM") as pnb,
            tc.tile_pool(name="singles", bufs=1) as singles,
            tc.tile_pool(name="small", bufs=2) as small,
        ):
            eye = singles.tile([P, P], f32, tag="eye")
            nc.sync.dma_start(out=eye, in_=eye_d[:])
            dot_buf = singles.tile([P, T], f32, tag="dot")
            na_buf = singles.tile([P, T], f32, tag="na")
            nb_buf = singles.tile([P, T], f32, tag="nb")
            cos_buf = singles.tile([P, T], f32, tag="cos")
            scr = singles.tile([P, P], f32, tag="scr")
            scr_dve = singles.tile([P, D], f32, tag="scr_dve")
            scr_act = singles.tile([P, D], f32, tag="scr_act")

            def diag(dst, psum):
                # dst[p] = sum_f psum[p, f] * eye[p, f] = psum[p, p]
                nc.vector.scalar_tensor_tensor(
                    out=scr,
                    in0=psum,
                    scalar=1.0,
                    in1=eye,
                    op0=Alu.mult,
                    op1=Alu.mult,
                    accum_out=dst,
                )

            if hw_loop and reps > 1:
                rep_ctx = tc.For_i(0, reps)
                rep_range = [0]
            else:
                rep_ctx = contextlib.nullcontext()
                rep_range = range(reps)

            with rep_ctx:
              for _rep in rep_range:
                for col, (path, idx) in enumerate(_ORDER):
                    if path == "pe":
                        abt = abpool.tile([P, K, 2, P], f8, tag="ab")
                        nc.sync.dma_start(out=abt, in_=abv[idx])
                        # Full-bank psum tiles ([128,512] f32 = 2 KiB/part)
                        # so each accumulation group owns its zero-region.
                        ps_ad = pad.tile([P, 512], f32, tag="ad")
                        ps_nb = pnb.tile([P, 512], f32, tag="nb")
                        for kp in range(KP):
                            sa = abt[:, 2 * kp : 2 * kp + 2, 0, :]
                            sb = abt[:, 2 * kp : 2 * kp + 2, 1, :]
                            sab = abt[:, 2 * kp : 2 * kp + 2, :, :]
                            first, last = kp == 0, kp == KP - 1
                            nc.tensor.matmul(
                                ps_ad[:, 0 : 2 * P],
                                sa,
                                sab,
                                start=first,
                                stop=last,
                                perf_mode=DR,
                            )
                            nc.tensor.matmul(
                                ps_nb[:, 0:P],
                                sb,
                                sb,
                                start=first,
                                stop=last,
                                perf_mode=DR,
                            )
                        diag(na_buf[:, col : col + 1], ps_ad[:, 0:P])
                        diag(dot_buf[:, col : col + 1], ps_ad[:, P : 2 * P])
                        diag(nb_buf[:, col : col + 1], ps_nb[:, 0:P])
                    else:
                        rmt = rmpool.tile([P, 2, D], f8, tag="rm")
                        nc.sync.dma_start(out=rmt, in_=rmv[idx])
                        nc.vector.scalar_tensor_tensor(
                            out=scr_dve,
                            in0=rmt[:, 0, :],
                            scalar=1.0,
                            in1=rmt[:, 1, :],
                            op0=Alu.mult,
                            op1=Alu.mult,
                            accum_out=dot_buf[:, col : col + 1],
                        )
                        nc.scalar.activation(
                            out=scr_act,
                            in_=rmt[:, 0, :],
                            func=Act.Square,
                            accum_out=na_buf[:, col : col + 1],
                        )
                        nc.scalar.activation(
                            out=scr_act,
                            in_=rmt[:, 1, :],
                            func=Act.Square,
                            accum_out=nb_buf[:, col : col + 1],
                        )

            # cos = dot / sqrt(na*nb), batched over all T columns
            prod = small.tile([P, T], f32, tag="prod")
            nc.vector.tensor_mul(prod, na_buf, nb_buf)
            rs = small.tile([P, T], f32, tag="rs")
            nc.scalar.sqrt(rs, prod)
            rr = small.tile([P, T], f32, tag="rr")
            nc.vector.reciprocal(rr, rs)
            nc.vector.tensor_mul(cos_buf, dot_buf, rr)
            nc.sync.dma_start(out=out[:], in_=cos_buf)

    _split_multi_waits(nc)
    return nc


def _get_nc():
    global _cached_nc
    if _cached_nc is None:
        _cached_nc = _build()
    return _cached_nc


def _run(in_maps, **kwargs):
    from concourse.bass_utils import run_bass_kernel_spmd

    return run_bass_kernel_spmd(
        _get_nc(), in_maps, core_ids=list(range(NCORES)), **kwargs
    )


def _interleave_pe(xa, xb):
    """[n*128 rows, D] fp8 pair -> [n*128, 2D] PE layout:
    row' = rb*128 + p, col = (k*2 + t)*128 + r."""
    n = xa.shape[0] // P
    xa = xa.reshape(n, P, K, P)  # [rb, r, k, p]  (row = rb*128+r, d = k*128+p)
    xb = xb.reshape(n, P, K, P)
    x = np.stack([xa, xb], axis=3)  # [rb, r, k, t, p]
    x = np.ascontiguousarray(x.transpose(0, 4, 2, 3, 1))  # [rb, p, k, t, r]
    return x.reshape(n * P, 2 * D)


def _interleave_rm(xa, xb):
    """[n*128 rows, D] fp8 pair -> [n*128, 2D] row-major, col = t*D + d."""
    x = np.stack([xa, xb], axis=1)  # [rows, t, d]
    return np.ascontiguousarray(x).reshape(xa.shape[0], 2 * D)


_PE_RB = [i for i, (p, _) in enumerate(_ORDER) if p == "pe"]
_RM_RB = [i for i, (p, _) in enumerate(_ORDER) if p == "rm"]


def _make_in_maps(cxr, ehr):
    cxr = np.asarray(cxr, dtype=np.float32).astype(ml_dtypes.float8_e4m3)
    ehr = np.asarray(ehr, dtype=np.float32).astype(ml_dtypes.float8_e4m3)
    eye = np.eye(P, dtype=np.float32)
    maps = []
    for i in range(NCORES):
        a = ehr[i * NS : (i + 1) * NS].reshape(T, P, D)
        b = cxr[i * NS : (i + 1) * NS].reshape(T, P, D)
        maps.append(
            {
                "ab": _interleave_pe(
                    a[_PE_RB].reshape(T_PE * P, D), b[_PE_RB].reshape(T_PE * P, D)
                ),
                "rm": _interleave_rm(
                    a[_RM_RB].reshape(RM * P, D), b[_RM_RB].reshape(RM * P, D)
                ),
                "eye": eye,
            }
        )
    return maps


def _combine(results):
    # cos[core, p, col]: by construction of _ORDER, output column col holds
    # the cosines of global rows core*2048 + col*128 + p.
    cos = np.stack([r["cos"] for r in results])  # [8, 128, 16]
    return np.float32(1.0 - cos.astype(np.float64).mean())


def kernel(cxr, ehr):
    res = _run(_make_in_maps(cxr, ehr))
    return _combine(res.results)
